# revision 3
# baseline (speedup 1.0000x reference)
"""GIN (3-layer) message-passing kernel for 8 Trainium2 NeuronCores.

Strategy (spmd, one program for all 8 cores):
  - 1D node partition: core c owns dst nodes [c*N/8, (c+1)*N/8).
  - Algebraic refactor: layer(h) = relu((h + A@h) @ W + b)
      with u = h @ W:  h_next = relu(u + A@u + b)  -- so the dense matmul
      runs on row-sharded data and the sparse aggregation A@u runs on the
      (replicated) u table via indirect-DMA row gathers.
  - 4 device launches:
      L1: u1 = x @ W1                       (row-sharded dense matmul)
      L2: h1 = relu(u1 + A@u1 + b1); u2 = h1 @ W2
      L3: h2 = relu(u2 + A@u2 + b2); u3 = h2 @ W3   (40 -> padded 64 cols)
      L4: h3 = u3 + A@u3 + b3; out_partial = onehot(batch)^T @ h3
    Host re-assembles/replicates u between launches (sharding glue only).
  - Aggregation: edges sorted by dst window (128 dst rows per PSUM tile).
    For each 128-edge tile: gather u[src] rows via gpsimd.dma_gather
    (int16 indices => the u table is split in 4 node quadrants), build
    S[e, r] = (r == dst_slot[e]) * w_e with one fused DVE tensor_scalar,
    and accumulate S^T @ gathered into the window's PSUM tile on TensorE.
"""

import numpy as np
import concourse.bass as bass
import concourse.mybir as mybir
import concourse.tile as tile
from concourse import bacc
from concourse.bass_utils import run_bass_kernel_spmd

F32 = mybir.dt.float32
I16 = mybir.dt.int16
AOT = mybir.AluOpType
ACT = mybir.ActivationFunctionType

NCORES = 8
WIN = 128           # dst rows per psum window
WB = 4              # windows per batch (psum tiles in flight)
CALL_TILES = 8      # max 128-edge tiles per dma_gather call (1024 idxs --
                    # dma_gather fails above 1024 idxs per call on HW)
SCRATCH = 16384     # dynamic dma scratch -> 1024-descriptor SWDGE ring


class Cfg:
    def __init__(self, N, E, IN=128, HID=128, C=40, G=64):
        assert N % (4 * NCORES) == 0
        self.N, self.E, self.IN, self.HID, self.C, self.G = N, E, IN, HID, C, G
        self.CP = 64                      # padded class count
        self.NPC = N // NCORES            # nodes per core
        self.NW = -(-self.NPC // WIN)     # windows per core
        self.NPAD = self.NW * WIN
        self.QROWS = N // 4               # nodes per quadrant table
        # one extra row (QROWS) holds the layer bias for the bias edges
        self.QPAD = -(-(self.QROWS + 1) // 128) * 128


class Plan:
    """Edge partition shared by all aggregation launches. Structure (tile
    counts / call layout) is identical across cores (padded to per-(w,q)
    max over cores); only the per-core data arrays differ.

    The GIN "+h" self term and "+b" bias are folded into the edge stream:
    one self-loop edge (w=1) per node, plus one edge (w=1) per node whose
    source is the bias row (row QROWS of each quadrant table, which the
    host fills with the layer bias)."""

    def __init__(self, cfg, src, dst, ew):
        self.cfg = cfg
        N, NPC, NW = cfg.N, cfg.NPC, cfg.NW
        nodes = np.arange(N, dtype=np.int64)
        # real edges + self loops
        src = np.concatenate([src, nodes])
        dst = np.concatenate([dst, nodes])
        ew = np.concatenate([ew.astype(np.float32), np.ones(N, np.float32)])
        core = dst // NPC
        dstl = dst - core * NPC
        w = dstl // WIN
        slot = dstl % WIN
        # table row order interleaves nodes mod 4 across quadrants so the
        # per-(window, quadrant) edge counts (incl. self loops) stay balanced
        q = src % 4
        srcl = src // 4

        cnt = np.zeros((NCORES, NW, 4), np.int64)
        np.add.at(cnt, (core, w, q), 1)
        T_wq = -(-cnt.max(axis=0) // 128)          # [NW, 4] tiles per group
        # bias edges (one per node, source = bias row of ANY quadrant) fill
        # leftover padding slots; bump a quadrant only if padding is short
        bias_need = np.minimum(NPC - np.arange(NW) * WIN, WIN)
        for ww in range(NW):
            while (T_wq[ww].sum() * 128 - cnt[:, ww, :].sum(axis=1).max()
                   < bias_need[ww]):
                T_wq[ww, int(cnt.max(axis=0)[ww].argmax())] += 1

        # stream order: batches of WB windows; inside a batch quadrant-major
        self.batches = []
        tile_w = []          # window-local id per tile
        tile_q = []
        self.calls = []      # (q, t0, ntiles) in stream order
        group_base = np.zeros((NW, 4), np.int64)   # first tile of group
        t_cursor = 0
        for b0 in range(0, NW, WB):
            b1 = min(b0 + WB, NW)
            c_lo = len(self.calls)
            t_lo = t_cursor
            for qq in range(4):
                run_t0 = t_cursor
                for ww in range(b0, b1):
                    group_base[ww, qq] = t_cursor
                    for _ in range(T_wq[ww, qq]):
                        tile_w.append(ww)
                        tile_q.append(qq)
                        t_cursor += 1
                # chunk this (batch, q) run into gather calls
                t = run_t0
                while t < t_cursor:
                    n = min(CALL_TILES, t_cursor - t)
                    self.calls.append((qq, t, n))
                    t += n
            self.batches.append((b0, b1, c_lo, len(self.calls), t_lo, t_cursor))
        self.NT = t_cursor
        self.tile_w = np.array(tile_w, np.int64)
        self.tile_q = np.array(tile_q, np.int64)

        # first/last tile of each window (for psum start/stop flags)
        self.w_first_tile = np.full(NW, -1, np.int64)
        self.w_last_tile = np.full(NW, -1, np.int64)
        self.w_ntiles = np.zeros(NW, np.int64)
        for t, ww in enumerate(tile_w):
            if self.w_first_tile[ww] < 0:
                self.w_first_tile[ww] = t
            self.w_last_tile[ww] = t
            self.w_ntiles[ww] += 1
        assert (self.w_ntiles > 0).all()

        # per-core padded data arrays (real + self edges by group rank)
        order = np.lexsort((q, w, core))           # edge order by (core,w,q)
        g_of_edge = (core * NW + w) * 4 + q
        gb_flat = group_base.reshape(-1)           # [NW*4]
        sorted_g = g_of_edge[order]
        starts = np.searchsorted(sorted_g, np.arange(NCORES * NW * 4))
        rank = np.arange(len(order)) - starts[sorted_g]
        pos = gb_flat[(w * 4 + q)] * 128 + rank[np.argsort(order, kind="stable")]
        # pos: slot position of each edge in its core's padded stream
        self.idx = np.zeros((NCORES, self.NT * 128), np.int16)
        self.slot = np.zeros((NCORES, self.NT * 128), np.float32)
        self.wgt = np.zeros((NCORES, self.NT * 128), np.float32)
        self.idx[core, pos] = srcl.astype(np.int16)
        self.slot[core, pos] = slot.astype(np.float32)
        self.wgt[core, pos] = ew.astype(np.float32)
        # bias edges into leftover padding slots (source row = QROWS)
        for c in range(NCORES):
            for ww in range(NW):
                need = int(bias_need[ww])
                bslots = []
                for qq in range(4):
                    lo = group_base[ww, qq] * 128 + cnt[c, ww, qq]
                    hi = (group_base[ww, qq] + T_wq[ww, qq]) * 128
                    take = min(need - len(bslots), hi - lo)
                    if take > 0:
                        bslots.extend(range(lo, lo + take))
                assert len(bslots) == need
                bs = np.array(bslots, np.int64)
                self.idx[c, bs] = cfg.QROWS
                self.slot[c, bs] = np.arange(need, dtype=np.float32)
                self.wgt[c, bs] = 1.0

    def idx_wrapped(self, c):
        # idx j -> partition j%16, col j//16; replicated to 128 partitions
        a = self.idx[c].reshape(-1, 16).T          # [16, NT*8]
        return np.ascontiguousarray(np.tile(a, (8, 1)))

    def col_arr(self, a, c):
        # [NT*128] -> [128, NT] (partition = position in tile)
        return np.ascontiguousarray(a[c].reshape(self.NT, 128).T)


def _iota_tile(n, m):
    return np.tile(np.arange(m, dtype=np.float32), (n, 1))


def build_l1(cfg):
    """u1 = x @ W1, row-sharded."""
    nc = bacc.Bacc("TRN2", target_bir_lowering=False, debug=False,
                   num_devices=NCORES, dynamic_dma_scratch_size=SCRATCH)
    x_d = nc.dram_tensor("x", [cfg.NPAD, cfg.IN], F32, kind="ExternalInput").ap()
    w_d = nc.dram_tensor("W", [cfg.IN, cfg.HID], F32, kind="ExternalInput").ap()
    id_d = nc.dram_tensor("ident", [128, 128], F32, kind="ExternalInput").ap()
    u_d = nc.dram_tensor("u", [cfg.NPAD, cfg.HID], F32, kind="ExternalOutput").ap()
    nt = cfg.NPAD // 128
    with tile.TileContext(nc) as tc:
        with tc.tile_pool(name="const", bufs=1) as cst, \
             tc.tile_pool(name="big", bufs=1) as big, \
             tc.tile_pool(name="io", bufs=8) as io, \
             tc.tile_pool(name="ps", bufs=4, space="PSUM") as ps:
            w_sb = cst.tile([cfg.IN, cfg.HID], F32)
            nc.sync.dma_start(out=w_sb[:], in_=w_d[:])
            id_sb = cst.tile([128, 128], F32)
            nc.sync.dma_start(out=id_sb[:], in_=id_d[:])
            x_sb = big.tile([128, nt, cfg.IN], F32)
            x_r = x_d.rearrange("(n p) d -> p n d", p=128)
            u_sb = big.tile([128, nt, cfg.HID], F32)
            u_r = u_d.rearrange("(n p) d -> p n d", p=128)
            CH = 14
            for lo in range(0, nt, CH):
                hi = min(lo + CH, nt)
                nc.sync.dma_start(out=x_sb[:, lo:hi, :], in_=x_r[:, lo:hi, :])
            for t in range(nt):
                xT_ps = ps.tile([128, 128], F32, tag="tp")
                nc.tensor.transpose(out=xT_ps[:], in_=x_sb[:, t, :],
                                    identity=id_sb[:])
                xT = io.tile([128, cfg.IN], F32, tag="xT")
                nc.scalar.activation(out=xT[:], in_=xT_ps[:], func=ACT.Copy)
                u_ps = ps.tile([128, cfg.HID], F32, tag="up")
                nc.tensor.matmul(out=u_ps[:], lhsT=xT[:], rhs=w_sb[:],
                                 start=True, stop=True)
                nc.vector.tensor_copy(out=u_sb[:, t, :], in_=u_ps[:])
                if (t + 1) % CH == 0 or t == nt - 1:
                    lo = (t // CH) * CH
                    nc.sync.dma_start(out=u_r[:, lo:t + 1, :],
                                      in_=u_sb[:, lo:t + 1, :])
    nc.compile()
    return nc


def build_agg(cfg, plan, feat, feat_out, relu, pool):
    """One aggregation launch:
         pre = u_agg + A@u + b       (per dst window, in PSUM)
         h   = relu(pre) if relu else pre
         if pool: out_partial += onehot(batch)^T @ h
         else:    u_out = h @ W
    """
    nc = bacc.Bacc("TRN2", target_bir_lowering=False, debug=False,
                   num_devices=NCORES, dynamic_dma_scratch_size=SCRATCH)
    uq = [nc.dram_tensor(f"uq{i}", [cfg.QPAD, feat], F32,
                         kind="ExternalInput").ap() for i in range(4)]
    id_d = nc.dram_tensor("ident", [128, 128], F32, kind="ExternalInput").ap()
    io_d = nc.dram_tensor("iota", [128, 128], F32, kind="ExternalInput").ap()
    ix_d = nc.dram_tensor("eidx", [128, plan.NT * 8], I16, kind="ExternalInput").ap()
    sl_d = nc.dram_tensor("eslot", [128, plan.NT], F32, kind="ExternalInput").ap()
    wg_d = nc.dram_tensor("ewgt", [128, plan.NT], F32, kind="ExternalInput").ap()
    if pool:
        ig_d = nc.dram_tensor("iotaG", [128, cfg.G], F32, kind="ExternalInput").ap()
        bv_d = nc.dram_tensor("bvals", [128, cfg.NW], F32, kind="ExternalInput").ap()
        out_d = nc.dram_tensor("pool", [cfg.G, cfg.CP], F32,
                               kind="ExternalOutput").ap()
    else:
        w_d = nc.dram_tensor("W", [feat, feat_out], F32, kind="ExternalInput").ap()
        out_d = nc.dram_tensor("u_out", [cfg.NPAD, feat_out], F32,
                               kind="ExternalOutput").ap()

    with tile.TileContext(nc) as tc:
        with tc.tile_pool(name="const", bufs=1) as cst, \
             tc.tile_pool(name="meta", bufs=1) as meta, \
             tc.tile_pool(name="gath", bufs=4) as gath, \
             tc.tile_pool(name="sp", bufs=12) as sp, \
             tc.tile_pool(name="io", bufs=6) as io, \
             tc.tile_pool(name="aps", bufs=6, space="PSUM") as aps, \
             tc.tile_pool(name="ops", bufs=1, space="PSUM") as ops:
            id_sb = cst.tile([128, 128], F32)
            nc.sync.dma_start(out=id_sb[:], in_=id_d[:])
            iota_sb = cst.tile([128, 128], F32)
            nc.sync.dma_start(out=iota_sb[:], in_=io_d[:])
            if pool:
                ig_sb = cst.tile([128, cfg.G], F32)
                nc.sync.dma_start(out=ig_sb[:], in_=ig_d[:])
                bv_sb = cst.tile([128, cfg.NW], F32)
                nc.sync.dma_start(out=bv_sb[:], in_=bv_d[:])
                pool_ps = ops.tile([cfg.G, cfg.CP], F32, tag="pool")
            else:
                w_sb = cst.tile([feat, feat_out], F32)
                nc.sync.dma_start(out=w_sb[:], in_=w_d[:])
            ix_sb = meta.tile([128, plan.NT * 8], I16)
            nc.sync.dma_start(out=ix_sb[:], in_=ix_d[:])
            sl_sb = meta.tile([128, plan.NT], F32)
            nc.sync.dma_start(out=sl_sb[:], in_=sl_d[:])
            wg_sb = meta.tile([128, plan.NT], F32)
            nc.sync.dma_start(out=wg_sb[:], in_=wg_d[:])

            for (b0, b1, c_lo, c_hi, t_lo, t_hi) in plan.batches:
                # gather calls for this batch
                tile_src = {}      # tile idx -> (sbuf tile, col)
                for ci in range(c_lo, c_hi):
                    qq, t0, ntl = plan.calls[ci]
                    gt = gath.tile([128, ntl, feat], F32, tag="gt")
                    nidx = ntl * 128
                    nc.gpsimd.dma_gather(
                        gt[:], uq[qq][:], ix_sb[:, t0 * 8:(t0 + ntl) * 8],
                        nidx, nidx, feat)
                    for k in range(ntl):
                        tile_src[t0 + k] = (gt, k)
                # psum tiles for the batch's windows
                wps = {ww: aps.tile([128, feat], F32, tag="agg",
                                    name=f"agg_{ww}")
                       for ww in range(b0, b1)}
                # edge-tile matmuls
                for t in range(t_lo, t_hi):
                    ww = int(plan.tile_w[t])
                    s_t = sp.tile([128, 128], F32, tag="S")
                    nc.vector.tensor_scalar(
                        out=s_t[:], in0=iota_sb[:],
                        scalar1=sl_sb[:, t:t + 1], scalar2=wg_sb[:, t:t + 1],
                        op0=AOT.is_equal, op1=AOT.mult)
                    gt, k = tile_src[t]
                    nc.tensor.matmul(
                        out=wps[ww][:], lhsT=s_t[:], rhs=gt[:, k, :],
                        start=(t == plan.w_first_tile[ww]),
                        stop=(t == plan.w_last_tile[ww]))
                # window tails
                for ww in range(b0, b1):
                    pw = wps[ww]
                    h_t = io.tile([128, feat], F32, tag="h")
                    if relu:
                        nc.scalar.activation(out=h_t[:], in_=pw[:], func=ACT.Relu)
                    else:
                        nc.vector.tensor_copy(out=h_t[:], in_=pw[:])
                    if pool:
                        s_p = sp.tile([128, cfg.G], F32, tag="SP")
                        nc.vector.tensor_scalar(
                            out=s_p[:], in0=ig_sb[:],
                            scalar1=bv_sb[:, ww:ww + 1], scalar2=None,
                            op0=AOT.is_equal)
                        nc.tensor.matmul(
                            out=pool_ps[:], lhsT=s_p[:], rhs=h_t[:],
                            start=(ww == 0), stop=(ww == cfg.NW - 1))
                    else:
                        tp = ops.tile([128, 128], F32, tag="tp")
                        nc.tensor.transpose(out=tp[:], in_=h_t[:],
                                            identity=id_sb[:])
                        hT = io.tile([128, 128], F32, tag="hT")
                        nc.vector.tensor_copy(out=hT[:], in_=tp[:])
                        up = ops.tile([128, feat_out], F32, tag="up")
                        nc.tensor.matmul(out=up[:], lhsT=hT[:], rhs=w_sb[:],
                                         start=True, stop=True)
                        uo = io.tile([128, feat_out], F32, tag="uo")
                        nc.vector.tensor_copy(out=uo[:], in_=up[:])
                        nc.sync.dma_start(
                            out=out_d[ww * 128:(ww + 1) * 128, :], in_=uo[:])
            if pool:
                po = io.tile([cfg.G, cfg.CP], F32, tag="po")
                nc.vector.tensor_copy(out=po[:], in_=pool_ps[:])
                nc.sync.dma_start(out=out_d[:], in_=po[:])
    nc.compile()
    return nc


def _pad_rows(a, rows):
    out = np.zeros((rows, a.shape[1]), np.float32)
    out[:a.shape[0]] = a
    return out


def _quad_tables(cfg, u_full, feat, bias):
    """Quadrant tables (interleaved: table q row r = node 4r+q) with the
    layer bias in row QROWS (the target of the bias edges)."""
    out = []
    for i in range(4):
        t = _pad_rows(np.ascontiguousarray(u_full[i::4]), cfg.QPAD)
        t[cfg.QROWS] = bias
        out.append(t)
    return out


TRACE = False
PROFILE = []        # (name, exec_time_ns, trace_path) per launch when TRACE


def _run(nc, in_maps, name=""):
    r = run_bass_kernel_spmd(nc, in_maps, core_ids=list(range(NCORES)),
                             trace=TRACE)
    if TRACE:
        tp = r.instructions_and_trace[1] if r.instructions_and_trace else None
        PROFILE.append((name, r.exec_time_ns, tp))
    return r.results


def _agg_inputs(cfg, plan, uqt):
    maps = []
    for c in range(NCORES):
        m = {f"uq{i}": uqt[i] for i in range(4)}
        m["ident"] = np.eye(128, dtype=np.float32)
        m["iota"] = _iota_tile(128, 128)
        m["eidx"] = plan.idx_wrapped(c)
        m["eslot"] = plan.col_arr(plan.slot, c)
        m["ewgt"] = plan.col_arr(plan.wgt, c)
        maps.append(m)
    return maps


def gin_forward(cfg, x, edge_index, edge_weight, batch,
                W1, b1, W2, b2, W3, b3, ncs=None):
    """ncs: optional dict of prebuilt programs {l1, l2, l3, l4}."""
    src = np.asarray(edge_index[0], np.int64)
    dst = np.asarray(edge_index[1], np.int64)
    plan = Plan(cfg, src, dst, np.asarray(edge_weight, np.float32))
    if ncs is None:
        ncs = {}
    if "l1" not in ncs:
        ncs["l1"] = build_l1(cfg)
        ncs["l2"] = build_agg(cfg, plan, cfg.HID, cfg.HID, True, False)
        ncs["l3"] = build_agg(cfg, plan, cfg.HID, cfg.CP, True, False)
        ncs["l4"] = build_agg(cfg, plan, cfg.CP, None, False, True)

    x = np.asarray(x, np.float32)
    ident = np.eye(128, dtype=np.float32)

    # L1: u1 = x @ W1
    maps = [{"x": _pad_rows(x[c * cfg.NPC:(c + 1) * cfg.NPC], cfg.NPAD),
             "W": np.asarray(W1, np.float32), "ident": ident}
            for c in range(NCORES)]
    res = _run(ncs["l1"], maps, "l1")
    u1 = np.concatenate([res[c]["u"][:cfg.NPC] for c in range(NCORES)])

    # L2: h1 = relu(u1 + A@u1 + b1); u2 = h1 @ W2
    maps = _agg_inputs(cfg, plan,
                       _quad_tables(cfg, u1, cfg.HID, np.asarray(b1)))
    for m in maps:
        m["W"] = np.asarray(W2, np.float32)
    res = _run(ncs["l2"], maps, "l2")
    u2 = np.concatenate([res[c]["u_out"][:cfg.NPC] for c in range(NCORES)])

    # L3: h2 = relu(u2 + A@u2 + b2); u3 = h2 @ W3pad
    W3p = np.zeros((cfg.HID, cfg.CP), np.float32)
    W3p[:, :cfg.C] = np.asarray(W3, np.float32)
    maps = _agg_inputs(cfg, plan,
                       _quad_tables(cfg, u2, cfg.HID, np.asarray(b2)))
    for m in maps:
        m["W"] = W3p
    res = _run(ncs["l3"], maps, "l3")
    u3 = np.concatenate([res[c]["u_out"][:cfg.NPC] for c in range(NCORES)])

    # L4: h3 = u3 + A@u3 + b3; pool
    b3p = np.zeros(cfg.CP, np.float32)
    b3p[:cfg.C] = np.asarray(b3, np.float32)
    batch64 = np.asarray(batch, np.int64)
    maps = _agg_inputs(cfg, plan, _quad_tables(cfg, u3, cfg.CP, b3p))
    for c, m in enumerate(maps):
        m["iotaG"] = _iota_tile(128, cfg.G)
        bv = np.full(cfg.NPAD, -1.0, np.float32)
        bv[:cfg.NPC] = batch64[c * cfg.NPC:(c + 1) * cfg.NPC].astype(np.float32)
        m["bvals"] = np.ascontiguousarray(bv.reshape(cfg.NW, 128).T)
    res = _run(ncs["l4"], maps, "l4")
    out = np.zeros((cfg.G, cfg.CP), np.float32)
    for c in range(NCORES):
        out += res[c]["pool"]
    return out[:, :cfg.C].astype(np.float32)


def kernel(x, edge_index, edge_weight, batch, W1, b1, W2, b2, W3, b3):
    cfg = Cfg(N=100000, E=1600000)
    return gin_forward(cfg, x, edge_index, edge_weight, batch,
                       W1, b1, W2, b2, W3, b3)



# revision 8
# speedup vs baseline: 1.6336x; 1.6336x over previous
"""GIN (3-layer) message-passing kernel for 8 Trainium2 NeuronCores.

Strategy (spmd, one program for all 8 cores):
  - 1D node partition: core c owns dst nodes [c*N/8, (c+1)*N/8).
  - Algebraic refactor: layer(h) = relu((h + A@h) @ W + b)
      with u = h @ W:  h_next = relu(u + A@u + b)  -- the dense matmul runs
      on row-sharded data; the sparse aggregation A@u gathers rows of the
      (replicated) u table via gpsimd dma_gather (4 SWDGE queues).
  - All tables / matmuls in bf16 (fp32 PSUM accumulate): halves gather
    bytes, enables FWL weight loads, doubles DVE throughput.
  - 4 device launches:
      L1: u1 = x @ W1                  (host pre-transposes x: no on-chip T)
      L2: h1 = relu(u1 + A@u1 + b1); u2 = h1 @ W2      (transposed agg)
      L3: h2 = relu(u2 + A@u2 + b2); u3 = h2 @ W3pad   (transposed agg)
      L4: h3 = u3 + A@u3; out_partial = onehot(batch)^T @ h3
          (+b3 and the 40-col slice are folded in on the host: adding
           count_g * b3 to the pooled sums is exact)
    Host re-assembles/replicates u between launches (sharding glue only).
  - Aggregation (L2/L3, transposed): for dst window w the PSUM tile is
    preT[feat=128, dst=128].  Self term u[dst] comes from the core's own
    u rows (SBUF-resident, sequential load) via one identity matmul; the
    layer bias is the per-partition bias of the Relu activation; so the
    edge stream holds REAL edges only.  Per 128-edge tile:
    S[e, d] = (d == dst_slot[e]) * w_e built by one DVE tensor_scalar,
    then matmul(out=preT, lhsT=gathered[e, feat], rhs=S[e, d]).
    Tail: hT = relu(preT + b) (ACT, per-partition bias), then
    matmul(out=u_out[dst, fo], lhsT=hT, rhs=W) -> DMA.
  - L4 runs the non-transposed orientation (pool needs dst on partitions).
  - Edge padding slots carry idx=-1 (dma_gather skips trailing negatives:
    no descriptor emitted) and S-weight 0; gather buffers are memset once
    so stale slots stay finite.  One dummy idx-0 slot per call keeps the
    completion semaphore alive when a core's call would otherwise be empty.
"""

import numpy as np
import ml_dtypes
import concourse.bass as bass
import concourse.mybir as mybir
import concourse.tile as tile
from concourse import bacc
from concourse.bass_utils import run_bass_kernel_spmd

F32 = mybir.dt.float32
BF16 = mybir.dt.bfloat16
I16 = mybir.dt.int16
AOT = mybir.AluOpType
ACT = mybir.ActivationFunctionType
BF = ml_dtypes.bfloat16

NCORES = 8
WIN = 128           # dst rows per psum window
WB = 4              # windows per batch
MAXCALL = 8         # max 128-edge tiles per dma_gather call (1024 idx limit)
NQ = 4              # SWDGE queues (ucode max)
SCRATCH = 32768     # dynamic dma scratch (descriptor rings)


class Cfg:
    def __init__(self, N, E, IN=128, HID=128, C=40, G=64):
        assert N % (4 * NCORES) == 0
        self.N, self.E, self.IN, self.HID, self.C, self.G = N, E, IN, HID, C, G
        self.CP = 128                     # padded class count (256B bf16 rows)
        self.NPC = N // NCORES            # nodes per core
        self.NW = -(-self.NPC // WIN)     # windows per core
        self.NPAD = self.NW * WIN
        self.QROWS = N // 4               # nodes per quadrant table
        self.QPAD = -(-self.QROWS // 128) * 128


class Plan:
    """Edge partition shared by all aggregation launches. Structure (tile
    counts / call layout) is identical across cores (padded to per-(w,q)
    max over cores); only the per-core data arrays differ."""

    def __init__(self, cfg, src, dst, ew):
        self.cfg = cfg
        N, NPC, NW = cfg.N, cfg.NPC, cfg.NW
        core = dst // NPC
        dstl = dst - core * NPC
        w = dstl // WIN
        slot = dstl % WIN
        # table row order interleaves nodes mod 4 across quadrants so the
        # per-(window, quadrant) edge counts stay balanced
        q = src % 4
        srcl = src // 4

        cnt = np.zeros((NCORES, NW, 4), np.int64)
        np.add.at(cnt, (core, w, q), 1)
        T_wq = -(-cnt.max(axis=0) // 128)          # [NW, 4] tiles per group
        T_wq = np.maximum(T_wq, 1)

        # stream order: batches of WB windows; inside a batch quadrant-major;
        # one gather call per (window, quadrant) group (split at MAXCALL)
        self.batches = []
        tile_w = []
        self.calls = []      # (q, t0, ntiles) in stream order
        group_base = np.zeros((NW, 4), np.int64)
        t_cursor = 0
        for b0 in range(0, NW, WB):
            b1 = min(b0 + WB, NW)
            c_lo = len(self.calls)
            t_lo = t_cursor
            for qq in range(4):
                for ww in range(b0, b1):
                    group_base[ww, qq] = t_cursor
                    g_tiles = int(T_wq[ww, qq])
                    tile_w.extend([ww] * g_tiles)
                    t = t_cursor
                    t_cursor += g_tiles
                    while t < t_cursor:
                        n = min(MAXCALL, t_cursor - t)
                        self.calls.append((qq, t, n))
                        t += n
            self.batches.append((b0, b1, c_lo, len(self.calls), t_lo, t_cursor))
        self.NT = t_cursor
        self.tile_w = np.array(tile_w, np.int64)

        self.w_first_tile = np.full(NW, -1, np.int64)
        self.w_last_tile = np.full(NW, -1, np.int64)
        for t, ww in enumerate(tile_w):
            if self.w_first_tile[ww] < 0:
                self.w_first_tile[ww] = t
            self.w_last_tile[ww] = t

        # per-core padded data arrays; padding gathers row 0 with weight 0
        # (negative "skip" indices crash the gather ucode on this runtime)
        order = np.lexsort((srcl, q, w, core))     # (core, w, q, src-sorted)
        g_of_edge = (core * NW + w) * 4 + q
        gb_flat = group_base.reshape(-1)
        sorted_g = g_of_edge[order]
        starts = np.searchsorted(sorted_g, np.arange(NCORES * NW * 4))
        rank = np.arange(len(order)) - starts[sorted_g]
        pos = gb_flat[(w * 4 + q)] * 128 + rank[np.argsort(order, kind="stable")]
        self.idx = np.zeros((NCORES, self.NT * 128), np.int16)
        self.slot = np.zeros((NCORES, self.NT * 128), np.float32)
        self.wgt = np.zeros((NCORES, self.NT * 128), np.float32)
        self.idx[core, pos] = srcl.astype(np.int16)
        self.slot[core, pos] = slot.astype(np.float32)
        self.wgt[core, pos] = ew.astype(np.float32)

    def idx_wrapped(self, c):
        a = self.idx[c].reshape(-1, 16).T          # [16, NT*8]
        return np.ascontiguousarray(np.tile(a, (8, 1)))

    def col_arr(self, a, c):
        return np.ascontiguousarray(a[c].reshape(self.NT, 128).T)


def _iota_tile(n, m):
    return np.tile(np.arange(m, dtype=np.float32), (n, 1))


def build_l1(cfg):
    """u1 = x @ W1, row-sharded; host supplies xT [IN, NPAD] bf16."""
    nc = bacc.Bacc("TRN2", target_bir_lowering=False, debug=False,
                   num_devices=NCORES, dynamic_dma_scratch_size=SCRATCH,
                   num_swdge_queues=NQ)
    xT_d = nc.dram_tensor("xT", [cfg.IN, cfg.NPAD], BF16, kind="ExternalInput").ap()
    w_d = nc.dram_tensor("W", [cfg.IN, cfg.HID], BF16, kind="ExternalInput").ap()
    u_d = nc.dram_tensor("u", [cfg.NPAD, cfg.HID], BF16, kind="ExternalOutput").ap()
    nt = cfg.NPAD // 128
    with tile.TileContext(nc) as tc:
        with tc.tile_pool(name="const", bufs=1) as cst, \
             tc.tile_pool(name="big", bufs=1) as big, \
             tc.tile_pool(name="io", bufs=6) as io, \
             tc.tile_pool(name="ps", bufs=6, space="PSUM") as ps:
            w_sb = cst.tile([cfg.IN, cfg.HID], BF16)
            nc.sync.dma_start(out=w_sb[:], in_=w_d[:])
            xT_sb = big.tile([128, cfg.NPAD], BF16)
            nc.sync.dma_start(out=xT_sb[:], in_=xT_d[:])
            u_sb = big.tile([128, nt, cfg.HID], BF16)
            u_r = u_d.rearrange("(n p) d -> p n d", p=128)
            CH = 14
            for t in range(nt):
                u_ps = ps.tile([128, cfg.HID], F32, tag="up")
                nc.tensor.matmul(out=u_ps[:], lhsT=xT_sb[:, t * 128:(t + 1) * 128],
                                 rhs=w_sb[:], start=True, stop=True)
                nc.scalar.activation(out=u_sb[:, t, :], in_=u_ps[:], func=ACT.Copy)
                if (t + 1) % CH == 0 or t == nt - 1:
                    lo = (t // CH) * CH
                    nc.sync.dma_start(out=u_r[:, lo:t + 1, :],
                                      in_=u_sb[:, lo:t + 1, :])
    nc.compile()
    return nc


def build_agg(cfg, plan, feat, feat_out, pool):
    """One aggregation launch.

    pool=False (L2/L3, transposed): per window preT[feat,dst] in PSUM =
      uown^T (self) + sum_tiles gathered^T S; tail hT=relu(preT+b) then
      u_out[dst,] = hT^T @ W -> DMA.
    pool=True (L4): per window pre[dst,feat] = uown + sum S^T gathered;
      tail pool_ps[G,feat] += onehot(batch_w)^T @ pre.
    """
    nc = bacc.Bacc("TRN2", target_bir_lowering=False, debug=False,
                   num_devices=NCORES, dynamic_dma_scratch_size=SCRATCH,
                   num_swdge_queues=NQ)
    uq = [nc.dram_tensor(f"uq{i}", [cfg.QPAD, feat], BF16,
                         kind="ExternalInput").ap() for i in range(4)]
    uo_d = nc.dram_tensor("uown", [128, cfg.NW * feat], BF16,
                          kind="ExternalInput").ap()
    id_d = nc.dram_tensor("ident", [128, 128], BF16, kind="ExternalInput").ap()
    io_d = nc.dram_tensor("iota", [128, 128], BF16, kind="ExternalInput").ap()
    ix_d = nc.dram_tensor("eidx", [128, plan.NT * 8], I16, kind="ExternalInput").ap()
    sl_d = nc.dram_tensor("eslot", [128, plan.NT], F32, kind="ExternalInput").ap()
    wg_d = nc.dram_tensor("ewgt", [128, plan.NT], F32, kind="ExternalInput").ap()
    if pool:
        ig_d = nc.dram_tensor("iotaG", [128, cfg.G], BF16, kind="ExternalInput").ap()
        bv_d = nc.dram_tensor("bvals", [128, cfg.NW], F32, kind="ExternalInput").ap()
        out_d = nc.dram_tensor("pool", [cfg.G, feat], F32,
                               kind="ExternalOutput").ap()
    else:
        b_d = nc.dram_tensor("bvec", [128, 1], F32, kind="ExternalInput").ap()
        w_d = nc.dram_tensor("W", [feat, feat_out], BF16, kind="ExternalInput").ap()
        out_d = nc.dram_tensor("u_out", [cfg.NPAD, feat_out], BF16,
                               kind="ExternalOutput").ap()

    with tile.TileContext(nc) as tc:
        with tc.tile_pool(name="const", bufs=1) as cst, \
             tc.tile_pool(name="meta", bufs=1) as meta, \
             tc.tile_pool(name="gath", bufs=20) as gath, \
             tc.tile_pool(name="sp", bufs=12) as sp, \
             tc.tile_pool(name="io", bufs=6) as io, \
             tc.tile_pool(name="aps", bufs=7 if pool else 5, space="PSUM") as aps, \
             tc.tile_pool(name="ops", bufs=1 if pool else 3, space="PSUM") as ops:
            id_sb = cst.tile([128, 128], BF16)
            nc.sync.dma_start(out=id_sb[:], in_=id_d[:])
            iota_sb = cst.tile([128, 128], BF16)
            nc.sync.dma_start(out=iota_sb[:], in_=io_d[:])
            uo_sb = cst.tile([128, cfg.NW * feat], BF16)
            nc.sync.dma_start(out=uo_sb[:], in_=uo_d[:])
            if pool:
                ig_sb = cst.tile([128, cfg.G], BF16)
                nc.sync.dma_start(out=ig_sb[:], in_=ig_d[:])
                bv_sb = cst.tile([128, cfg.NW], F32)
                nc.sync.dma_start(out=bv_sb[:], in_=bv_d[:])
                pool_ps = ops.tile([cfg.G, feat], F32, tag="pool")
            else:
                b_sb = cst.tile([128, 1], F32)
                nc.sync.dma_start(out=b_sb[:], in_=b_d[:])
                w_sb = cst.tile([feat, feat_out], BF16)
                nc.sync.dma_start(out=w_sb[:], in_=w_d[:])
            ix_sb = meta.tile([128, plan.NT * 8], I16)
            nc.sync.dma_start(out=ix_sb[:], in_=ix_d[:])
            sl_sb = meta.tile([128, plan.NT], F32)
            nc.sync.dma_start(out=sl_sb[:], in_=sl_d[:])
            wg_sb = meta.tile([128, plan.NT], F32)
            nc.sync.dma_start(out=wg_sb[:], in_=wg_d[:])

            # memset the gather slots once: idx=-1 padding leaves stale
            # bytes, which must be finite for the x0 matmul columns
            for _ in range(20):
                g0 = gath.tile([128, MAXCALL, feat], BF16, tag="gt")
                nc.vector.memset(g0[:], 0.0)

            for (b0, b1, c_lo, c_hi, t_lo, t_hi) in plan.batches:
                tile_src = {}
                for ci in range(c_lo, c_hi):
                    qq, t0, ntl = plan.calls[ci]
                    gt = gath.tile([128, ntl, feat], BF16, tag="gt")
                    nidx = ntl * 128
                    nc.gpsimd.dma_gather(
                        gt[:], uq[qq][:],
                        ix_sb[:, t0 * 8:(t0 + ntl) * 8],
                        nidx, nidx, feat, queue_num=ci % NQ)
                    for k in range(ntl):
                        tile_src[t0 + k] = (gt, k)
                wps = {ww: aps.tile([128, feat] if pool else [feat, 128],
                                    F32, tag="agg", name=f"agg_{ww}")
                       for ww in range(b0, b1)}
                # self term: pre(T) initialized from the core's own u rows
                for ww in range(b0, b1):
                    uo_w = uo_sb[:, ww * feat:(ww + 1) * feat]
                    if pool:
                        nc.tensor.matmul(out=wps[ww][:], lhsT=id_sb[:],
                                         rhs=uo_w, start=True,
                                         stop=(plan.w_first_tile[ww] < 0))
                    else:
                        nc.tensor.matmul(out=wps[ww][:], lhsT=uo_w,
                                         rhs=id_sb[:], start=True,
                                         stop=(plan.w_first_tile[ww] < 0))
                # edge-tile matmuls
                for t in range(t_lo, t_hi):
                    ww = int(plan.tile_w[t])
                    s_t = sp.tile([128, 128], BF16, tag="S")
                    nc.vector.tensor_scalar(
                        out=s_t[:], in0=iota_sb[:],
                        scalar1=sl_sb[:, t:t + 1], scalar2=wg_sb[:, t:t + 1],
                        op0=AOT.is_equal, op1=AOT.mult)
                    gt, k = tile_src[t]
                    stop = (t == plan.w_last_tile[ww])
                    if pool:
                        nc.tensor.matmul(out=wps[ww][:], lhsT=s_t[:],
                                         rhs=gt[:, k, :], start=False, stop=stop)
                    else:
                        nc.tensor.matmul(out=wps[ww][:], lhsT=gt[:, k, :],
                                         rhs=s_t[:], start=False, stop=stop)
                # window tails
                for ww in range(b0, b1):
                    pw = wps[ww]
                    if pool:
                        h_t = io.tile([128, feat], BF16, tag="h")
                        nc.scalar.activation(out=h_t[:], in_=pw[:], func=ACT.Copy)
                        s_p = sp.tile([128, cfg.G], BF16, tag="SP")
                        nc.vector.tensor_scalar(
                            out=s_p[:], in0=ig_sb[:],
                            scalar1=bv_sb[:, ww:ww + 1], scalar2=None,
                            op0=AOT.is_equal)
                        nc.tensor.matmul(
                            out=pool_ps[:], lhsT=s_p[:], rhs=h_t[:],
                            start=(ww == 0), stop=(ww == cfg.NW - 1))
                    else:
                        hT = io.tile([feat, 128], BF16, tag="h")
                        nc.scalar.activation(out=hT[:], in_=pw[:],
                                             func=ACT.Relu, bias=b_sb[:])
                        up = ops.tile([128, feat_out], F32, tag="up")
                        nc.tensor.matmul(out=up[:], lhsT=hT[:], rhs=w_sb[:],
                                         start=True, stop=True)
                        uo = io.tile([128, feat_out], BF16, tag="uo")
                        nc.scalar.activation(out=uo[:], in_=up[:], func=ACT.Copy)
                        nc.sync.dma_start(
                            out=out_d[ww * 128:(ww + 1) * 128, :], in_=uo[:])
            if pool:
                po = io.tile([cfg.G, feat], F32, tag="po")
                nc.vector.tensor_copy(out=po[:], in_=pool_ps[:])
                nc.sync.dma_start(out=out_d[:], in_=po[:])
    nc.compile()
    return nc


def _pad_rows(a, rows):
    out = np.zeros((rows, a.shape[1]), a.dtype)
    out[:a.shape[0]] = a
    return out


def _quad_tables(cfg, u_full):
    """Quadrant tables (interleaved: table q row r = node 4r+q), bf16."""
    return [_pad_rows(np.ascontiguousarray(u_full[i::4]), cfg.QPAD)
            for i in range(4)]


def _uown(cfg, u_full, feat, c):
    """[128, NW*feat]: partition p, cols w*feat.. = row (w*128+p) of the
    core's shard."""
    sh = _pad_rows(u_full[c * cfg.NPC:(c + 1) * cfg.NPC], cfg.NPAD)
    return np.ascontiguousarray(
        sh.reshape(cfg.NW, 128, feat).transpose(1, 0, 2).reshape(128, -1))


TRACE = False
PROFILE = []        # (name, exec_time_ns, trace_path) per launch when TRACE


def _run(nc, in_maps, name=""):
    r = run_bass_kernel_spmd(nc, in_maps, core_ids=list(range(NCORES)),
                             trace=TRACE)
    if TRACE:
        tp = r.instructions_and_trace[1] if r.instructions_and_trace else None
        PROFILE.append((name, r.exec_time_ns, tp))
    return r.results


def _agg_inputs(cfg, plan, u_full, feat):
    uqt = _quad_tables(cfg, u_full)
    maps = []
    for c in range(NCORES):
        m = {f"uq{i}": uqt[i] for i in range(4)}
        m["uown"] = _uown(cfg, u_full, feat, c)
        m["ident"] = np.eye(128, dtype=BF)
        m["iota"] = _iota_tile(128, 128).astype(BF)
        m["eidx"] = plan.idx_wrapped(c)
        m["eslot"] = plan.col_arr(plan.slot, c)
        m["ewgt"] = plan.col_arr(plan.wgt, c)
        maps.append(m)
    return maps


def gin_forward(cfg, x, edge_index, edge_weight, batch,
                W1, b1, W2, b2, W3, b3, ncs=None):
    src = np.asarray(edge_index[0], np.int64)
    dst = np.asarray(edge_index[1], np.int64)
    plan = Plan(cfg, src, dst, np.asarray(edge_weight, np.float32))
    if ncs is None:
        ncs = {}
    if "l1" not in ncs:
        ncs["l1"] = build_l1(cfg)
        ncs["l2"] = build_agg(cfg, plan, cfg.HID, cfg.HID, False)
        ncs["l3"] = build_agg(cfg, plan, cfg.HID, cfg.CP, False)
        ncs["l4"] = build_agg(cfg, plan, cfg.CP, None, True)

    x = np.asarray(x, np.float32)

    # L1: u1 = x @ W1
    W1b = np.asarray(W1, np.float32).astype(BF)
    maps = [{"xT": np.ascontiguousarray(
                _pad_rows(x[c * cfg.NPC:(c + 1) * cfg.NPC], cfg.NPAD)
                .T.astype(BF)),
             "W": W1b} for c in range(NCORES)]
    res = _run(ncs["l1"], maps, "l1")
    u1 = np.concatenate([res[c]["u"][:cfg.NPC] for c in range(NCORES)])

    # L2: h1 = relu(u1 + A@u1 + b1); u2 = h1 @ W2
    maps = _agg_inputs(cfg, plan, u1, cfg.HID)
    for m in maps:
        m["W"] = np.asarray(W2, np.float32).astype(BF)
        m["bvec"] = np.asarray(b1, np.float32).reshape(128, 1)
    res = _run(ncs["l2"], maps, "l2")
    u2 = np.concatenate([res[c]["u_out"][:cfg.NPC] for c in range(NCORES)])

    # L3: h2 = relu(u2 + A@u2 + b2); u3 = h2 @ W3pad
    W3p = np.zeros((cfg.HID, cfg.CP), np.float32)
    W3p[:, :cfg.C] = np.asarray(W3, np.float32)
    maps = _agg_inputs(cfg, plan, u2, cfg.HID)
    for m in maps:
        m["W"] = W3p.astype(BF)
        m["bvec"] = np.asarray(b2, np.float32).reshape(128, 1)
    res = _run(ncs["l3"], maps, "l3")
    u3 = np.concatenate([res[c]["u_out"][:cfg.NPC] for c in range(NCORES)])

    # L4: h3 = u3 + A@u3; pool (b3 folded in on host below)
    batch64 = np.asarray(batch, np.int64)
    maps = _agg_inputs(cfg, plan, u3, cfg.CP)
    for c, m in enumerate(maps):
        m["iotaG"] = _iota_tile(128, cfg.G).astype(BF)
        bv = np.full(cfg.NPAD, -1.0, np.float32)
        bv[:cfg.NPC] = batch64[c * cfg.NPC:(c + 1) * cfg.NPC].astype(np.float32)
        m["bvals"] = np.ascontiguousarray(bv.reshape(cfg.NW, 128).T)
    res = _run(ncs["l4"], maps, "l4")
    out = np.zeros((cfg.G, cfg.CP), np.float32)
    for c in range(NCORES):
        out += res[c]["pool"]
    counts = np.bincount(batch64, minlength=cfg.G).astype(np.float32)
    out = out[:, :cfg.C] + counts[:, None] * np.asarray(b3, np.float32)[None, :]
    return out.astype(np.float32)


def kernel(x, edge_index, edge_weight, batch, W1, b1, W2, b2, W3, b3):
    cfg = Cfg(N=100000, E=1600000)
    return gin_forward(cfg, x, edge_index, edge_weight, batch,
                       W1, b1, W2, b2, W3, b3)


# revision 14
# speedup vs baseline: 2.2279x; 1.3638x over previous
"""GIN (3-layer) message-passing kernel for 8 Trainium2 NeuronCores.

Strategy (spmd, one program for all 8 cores):
  - 1D node partition: core c owns dst nodes [c*N/8, (c+1)*N/8).
  - Algebraic refactor: layer(h) = relu((h + A@h) @ W + b)
      with u = h @ W:  h_next = relu(u + A@u + b)  -- the dense matmul runs
      on row-sharded data; the sparse aggregation A@u gathers rows of the
      (replicated) u table via gpsimd dma_gather (4 SWDGE queues).
  - All tables / matmuls in bf16 (fp32 PSUM accumulate): halves gather
    bytes, enables FWL weight loads, doubles DVE throughput.
  - 4 device launches:
      L1: u1 = x @ W1                  (host pre-transposes x: no on-chip T)
      L2: h1 = relu(u1 + A@u1 + b1); u2 = h1 @ W2      (transposed agg)
      L3: h2 = relu(u2 + A@u2 + b2); u3 = h2 @ W3pad   (transposed agg)
      L4: h3 = u3 + A@u3; out_partial = onehot(batch)^T @ h3
          (+b3 and the 40-col slice are folded in on the host: adding
           count_g * b3 to the pooled sums is exact)
    Host re-assembles/replicates u between launches (sharding glue only).
  - Aggregation (L2/L3, transposed): for dst window w the PSUM tile is
    preT[feat=128, dst=128].  Self term u[dst] comes from the core's own
    u rows (SBUF-resident, sequential load) via one identity matmul; the
    layer bias is the per-partition bias of the Relu activation; so the
    edge stream holds REAL edges only.  Per 128-edge tile:
    S[e, d] = (d == dst_slot[e]) * w_e built by one DVE tensor_scalar,
    then matmul(out=preT, lhsT=gathered[e, feat], rhs=S[e, d]).
    Tail: hT = relu(preT + b) (ACT, per-partition bias), then
    matmul(out=u_out[dst, fo], lhsT=hT, rhs=W) -> DMA.
  - L4 runs the non-transposed orientation (pool needs dst on partitions).
  - Edge padding slots carry idx=-1 (dma_gather skips trailing negatives:
    no descriptor emitted) and S-weight 0; gather buffers are memset once
    so stale slots stay finite.  One dummy idx-0 slot per call keeps the
    completion semaphore alive when a core's call would otherwise be empty.
"""

import numpy as np
import ml_dtypes
import concourse.bass as bass
import concourse.mybir as mybir
import concourse.tile as tile
from concourse import bacc
from concourse.bass_utils import run_bass_kernel_spmd

F32 = mybir.dt.float32
BF16 = mybir.dt.bfloat16
I16 = mybir.dt.int16
AOT = mybir.AluOpType
ACT = mybir.ActivationFunctionType
BF = ml_dtypes.bfloat16

NCORES = 8
WIN = 128           # dst rows per psum window
WB = 4              # windows per batch
MAXCALL = 8         # max 128-edge tiles per dma_gather call (1024 idx limit)
NQ = 4              # SWDGE queues (ucode max)
SCRATCH = 32768     # dynamic dma scratch (descriptor rings)


class Cfg:
    def __init__(self, N, E, IN=128, HID=128, C=40, G=64):
        assert N % (4 * NCORES) == 0
        self.N, self.E, self.IN, self.HID, self.C, self.G = N, E, IN, HID, C, G
        self.CP = 128                     # padded class count (256B bf16 rows)
        self.NPC = N // NCORES            # nodes per core
        self.NW = -(-self.NPC // WIN)     # windows per core
        self.NPAD = self.NW * WIN
        self.QROWS = N // 4               # nodes per quadrant table
        self.QPAD = -(-self.QROWS // 128) * 128


class Plan:
    """Edge partition shared by all aggregation launches. Structure (tile
    counts / call layout) is identical across cores (padded to per-(w,q)
    max over cores); only the per-core data arrays differ."""

    def __init__(self, cfg, src, dst, ew):
        self.cfg = cfg
        N, NPC, NW = cfg.N, cfg.NPC, cfg.NW
        core = dst // NPC
        dstl = dst - core * NPC
        w = dstl // WIN
        slot = dstl % WIN
        # table row order interleaves nodes mod 4 across quadrants so the
        # per-(window, quadrant) edge counts stay balanced
        q = src % 4
        srcl = src // 4

        cnt = np.zeros((NCORES, NW, 4), np.int64)
        np.add.at(cnt, (core, w, q), 1)
        T_wq = -(-cnt.max(axis=0) // 128)          # [NW, 4] tiles per group
        T_wq = np.maximum(T_wq, 1)

        # stream order: batches of WB windows; inside a batch quadrant-major;
        # one gather call per (window, quadrant) group (split at MAXCALL)
        self.batches = []
        tile_w = []
        self.calls = []      # (q, t0, ntiles) in stream order
        group_base = np.zeros((NW, 4), np.int64)
        t_cursor = 0
        for b0 in range(0, NW, WB):
            b1 = min(b0 + WB, NW)
            c_lo = len(self.calls)
            t_lo = t_cursor
            for qq in range(4):
                for ww in range(b0, b1):
                    group_base[ww, qq] = t_cursor
                    g_tiles = int(T_wq[ww, qq])
                    tile_w.extend([ww] * g_tiles)
                    t = t_cursor
                    t_cursor += g_tiles
                    while t < t_cursor:
                        n = min(MAXCALL, t_cursor - t)
                        self.calls.append((qq, t, n))
                        t += n
            self.batches.append((b0, b1, c_lo, len(self.calls), t_lo, t_cursor))
        self.NT = t_cursor
        self.tile_w = np.array(tile_w, np.int64)

        self.w_first_tile = np.full(NW, -1, np.int64)
        self.w_last_tile = np.full(NW, -1, np.int64)
        for t, ww in enumerate(tile_w):
            if self.w_first_tile[ww] < 0:
                self.w_first_tile[ww] = t
            self.w_last_tile[ww] = t

        # per-core padded data arrays; padding gathers row 0 with weight 0
        # (negative "skip" indices crash the gather ucode on this runtime)
        order = np.lexsort((srcl, q, w, core))     # (core, w, q, src-sorted)
        g_of_edge = (core * NW + w) * 4 + q
        gb_flat = group_base.reshape(-1)
        sorted_g = g_of_edge[order]
        starts = np.searchsorted(sorted_g, np.arange(NCORES * NW * 4))
        rank = np.arange(len(order)) - starts[sorted_g]
        pos = gb_flat[(w * 4 + q)] * 128 + rank[np.argsort(order, kind="stable")]
        self.idx = np.zeros((NCORES, self.NT * 128), np.int16)
        self.slot = np.zeros((NCORES, self.NT * 128), np.float32)
        self.wgt = np.zeros((NCORES, self.NT * 128), np.float32)
        self.idx[core, pos] = srcl.astype(np.int16)
        self.slot[core, pos] = slot.astype(np.float32)
        self.wgt[core, pos] = ew.astype(np.float32)

    def idx_wrapped(self, c):
        a = self.idx[c].reshape(-1, 16).T          # [16, NT*8]
        return np.ascontiguousarray(np.tile(a, (8, 1)))

    def col_arr(self, a, c):
        return np.ascontiguousarray(a[c].reshape(self.NT, 128).T)


def _iota_tile(n, m):
    return np.tile(np.arange(m, dtype=np.float32), (n, 1))


def build_l1(cfg):
    """u1 = x @ W1, row-sharded; host supplies xT [IN, NPAD] bf16."""
    nc = bacc.Bacc("TRN2", target_bir_lowering=False, debug=False,
                   num_devices=NCORES, dynamic_dma_scratch_size=SCRATCH,
                   num_swdge_queues=NQ)
    xT_d = nc.dram_tensor("xT", [cfg.IN, cfg.NPAD], BF16, kind="ExternalInput").ap()
    w_d = nc.dram_tensor("W", [cfg.IN, cfg.HID], BF16, kind="ExternalInput").ap()
    u_d = nc.dram_tensor("u", [cfg.NPAD, cfg.HID], BF16, kind="ExternalOutput").ap()
    nt = cfg.NPAD // 128
    with tile.TileContext(nc) as tc:
        with tc.tile_pool(name="const", bufs=1) as cst, \
             tc.tile_pool(name="big", bufs=1) as big, \
             tc.tile_pool(name="io", bufs=6) as io, \
             tc.tile_pool(name="ps", bufs=6, space="PSUM") as ps:
            w_sb = cst.tile([cfg.IN, cfg.HID], BF16)
            nc.sync.dma_start(out=w_sb[:], in_=w_d[:])
            xT_sb = big.tile([128, cfg.NPAD], BF16)
            nc.sync.dma_start(out=xT_sb[:], in_=xT_d[:])
            u_sb = big.tile([128, nt, cfg.HID], BF16)
            u_r = u_d.rearrange("(n p) d -> p n d", p=128)
            CH = 14
            for t in range(nt):
                u_ps = ps.tile([128, cfg.HID], F32, tag="up")
                nc.tensor.matmul(out=u_ps[:], lhsT=xT_sb[:, t * 128:(t + 1) * 128],
                                 rhs=w_sb[:], start=True, stop=True)
                nc.scalar.activation(out=u_sb[:, t, :], in_=u_ps[:], func=ACT.Copy)
                if (t + 1) % CH == 0 or t == nt - 1:
                    lo = (t // CH) * CH
                    nc.sync.dma_start(out=u_r[:, lo:t + 1, :],
                                      in_=u_sb[:, lo:t + 1, :])
    nc.compile()
    return nc


def build_agg(cfg, plan, feat, feat_out, pool):
    """One aggregation launch.

    pool=False (L2/L3, transposed): per window preT[feat,dst] in PSUM =
      uown^T (self) + sum_tiles gathered^T S; tail hT=relu(preT+b) then
      u_out[dst,] = hT^T @ W -> DMA.
    pool=True (L4): per window pre[dst,feat] = uown + sum S^T gathered;
      tail pool_ps[G,feat] += onehot(batch_w)^T @ pre.
    """
    nc = bacc.Bacc("TRN2", target_bir_lowering=False, debug=False,
                   num_devices=NCORES, dynamic_dma_scratch_size=SCRATCH,
                   num_swdge_queues=NQ)
    uq = [nc.dram_tensor(f"uq{i}", [cfg.QPAD, feat], BF16,
                         kind="ExternalInput").ap() for i in range(4)]
    uo_d = nc.dram_tensor("uown", [128, cfg.NW * feat], BF16,
                          kind="ExternalInput").ap()
    id_d = nc.dram_tensor("ident", [128, 128], BF16, kind="ExternalInput").ap()
    io_d = nc.dram_tensor("iota", [128, 128], BF16, kind="ExternalInput").ap()
    ix_d = nc.dram_tensor("eidx", [128, plan.NT * 8], I16, kind="ExternalInput").ap()
    sl_d = nc.dram_tensor("eslot", [128, plan.NT], F32, kind="ExternalInput").ap()
    wg_d = nc.dram_tensor("ewgt", [128, plan.NT], F32, kind="ExternalInput").ap()
    nsl_d = nc.dram_tensor("enslot", [128, plan.NT], F32, kind="ExternalInput").ap()
    nwg_d = nc.dram_tensor("enwgt", [128, plan.NT], F32, kind="ExternalInput").ap()
    if pool:
        ig_d = nc.dram_tensor("iotaG", [128, cfg.G], BF16, kind="ExternalInput").ap()
        bv_d = nc.dram_tensor("bvals", [128, cfg.NW], F32, kind="ExternalInput").ap()
        out_d = nc.dram_tensor("pool", [cfg.G, feat], F32,
                               kind="ExternalOutput").ap()
    else:
        b_d = nc.dram_tensor("bvec", [128, 1], F32, kind="ExternalInput").ap()
        w_d = nc.dram_tensor("W", [feat, feat_out], BF16, kind="ExternalInput").ap()
        out_d = nc.dram_tensor("u_out", [cfg.NPAD, feat_out], BF16,
                               kind="ExternalOutput").ap()

    with tile.TileContext(nc) as tc:
        with tc.tile_pool(name="const", bufs=1) as cst, \
             tc.tile_pool(name="meta", bufs=1) as meta, \
             tc.tile_pool(name="gath", bufs=20) as gath, \
             tc.tile_pool(name="sp", bufs=12) as sp, \
             tc.tile_pool(name="io", bufs=6) as io, \
             tc.tile_pool(name="aps", bufs=6 if pool else 5, space="PSUM") as aps, \
             tc.tile_pool(name="ips", bufs=1, space="PSUM") as ips, \
             tc.tile_pool(name="ops", bufs=1 if pool else 2, space="PSUM") as ops:
            id_sb = cst.tile([128, 128], BF16)
            nc.sync.dma_start(out=id_sb[:], in_=id_d[:])
            iota_sb = cst.tile([128, 128], BF16)
            nc.sync.dma_start(out=iota_sb[:], in_=io_d[:])
            uo_sb = cst.tile([128, cfg.NW * feat], BF16)
            nc.sync.dma_start(out=uo_sb[:], in_=uo_d[:])
            if pool:
                ig_sb = cst.tile([128, cfg.G], BF16)
                nc.sync.dma_start(out=ig_sb[:], in_=ig_d[:])
                bv_sb = cst.tile([128, cfg.NW], F32)
                nc.sync.dma_start(out=bv_sb[:], in_=bv_d[:])
                pool_ps = ops.tile([cfg.G, feat], F32, tag="pool")
            else:
                b_sb = cst.tile([128, 1], F32)
                nc.sync.dma_start(out=b_sb[:], in_=b_d[:])
                w_sb = cst.tile([feat, feat_out], BF16)
                nc.sync.dma_start(out=w_sb[:], in_=w_d[:])
            ix_sb = meta.tile([128, plan.NT * 8], I16)
            nc.sync.dma_start(out=ix_sb[:], in_=ix_d[:])
            sl_sb = meta.tile([128, plan.NT], F32)
            nc.sync.dma_start(out=sl_sb[:], in_=sl_d[:])
            wg_sb = meta.tile([128, plan.NT], F32)
            nc.sync.dma_start(out=wg_sb[:], in_=wg_d[:])
            nsl_sb = meta.tile([128, plan.NT], F32)
            nc.sync.dma_start(out=nsl_sb[:], in_=nsl_d[:])
            nwg_sb = meta.tile([128, plan.NT], F32)
            nc.sync.dma_start(out=nwg_sb[:], in_=nwg_d[:])

            # iota in PSUM: a DVE tensor_scalar whose input is PSUM is not
            # eligible for the 2-port SBUF perf mode, which otherwise
            # serializes against SWDGE descriptor emission (5us stalls)
            one_sb = cst.tile([1, 128], BF16)
            nc.vector.memset(one_sb[:], 1.0)
            ip = ips.tile([128, 128], F32, tag="iotaps")
            nc.tensor.matmul(out=ip[:], lhsT=one_sb[:], rhs=iota_sb[0:1, :],
                             start=True, stop=True)

            for (b0, b1, c_lo, c_hi, t_lo, t_hi) in plan.batches:
                tile_src = {}
                for ci in range(c_lo, c_hi):
                    qq, t0, ntl = plan.calls[ci]
                    gt = gath.tile([128, ntl, feat], BF16, tag="gt")
                    nidx = ntl * 128
                    nc.gpsimd.dma_gather(
                        gt[:], uq[qq][:],
                        ix_sb[:, t0 * 8:(t0 + ntl) * 8],
                        nidx, nidx, feat, queue_num=ci % NQ)
                    for k in range(ntl):
                        tile_src[t0 + k] = (gt, k)
                wps = {ww: aps.tile([128, feat] if pool else [feat, 128],
                                    F32, tag="agg", name=f"agg_{ww}")
                       for ww in range(b0, b1)}
                # self term: pre(T) initialized from the core's own u rows
                for ww in range(b0, b1):
                    uo_w = uo_sb[:, ww * feat:(ww + 1) * feat]
                    if pool:
                        nc.tensor.matmul(out=wps[ww][:], lhsT=id_sb[:],
                                         rhs=uo_w, start=True,
                                         stop=(plan.w_first_tile[ww] < 0))
                    else:
                        nc.tensor.matmul(out=wps[ww][:], lhsT=uo_w,
                                         rhs=id_sb[:], start=True,
                                         stop=(plan.w_first_tile[ww] < 0))
                # edge-tile matmuls; S built on DVE (2/3, PSUM-input iota)
                # and ACT (1/3: wgt*relu(1-|iota-slot|) via two activations)
                for t in range(t_lo, t_hi):
                    ww = int(plan.tile_w[t])
                    s_t = sp.tile([128, 128], BF16, tag="S")
                    if t % 3 == 2:
                        a_t = sp.tile([128, 128], BF16, tag="A")
                        nc.scalar.activation(
                            out=a_t[:], in_=ip[:], func=ACT.Abs,
                            bias=nsl_sb[:, t:t + 1])
                        nc.scalar.activation(
                            out=s_t[:], in_=a_t[:], func=ACT.Relu,
                            bias=wg_sb[:, t:t + 1],
                            scale=nwg_sb[:, t:t + 1])
                    else:
                        nc.vector.tensor_scalar(
                            out=s_t[:], in0=ip[:],
                            scalar1=sl_sb[:, t:t + 1], scalar2=wg_sb[:, t:t + 1],
                            op0=AOT.is_equal, op1=AOT.mult)
                    gt, k = tile_src[t]
                    stop = (t == plan.w_last_tile[ww])
                    if pool:
                        nc.tensor.matmul(out=wps[ww][:], lhsT=s_t[:],
                                         rhs=gt[:, k, :], start=False, stop=stop)
                    else:
                        nc.tensor.matmul(out=wps[ww][:], lhsT=gt[:, k, :],
                                         rhs=s_t[:], start=False, stop=stop)
                # window tails
                for ww in range(b0, b1):
                    pw = wps[ww]
                    if pool:
                        h_t = io.tile([128, feat], BF16, tag="h")
                        nc.scalar.activation(out=h_t[:], in_=pw[:], func=ACT.Copy)
                        s_p = sp.tile([128, cfg.G], BF16, tag="SP")
                        nc.vector.tensor_scalar(
                            out=s_p[:], in0=ip[:, :cfg.G],
                            scalar1=bv_sb[:, ww:ww + 1], scalar2=None,
                            op0=AOT.is_equal)
                        nc.tensor.matmul(
                            out=pool_ps[:], lhsT=s_p[:], rhs=h_t[:],
                            start=(ww == 0), stop=(ww == cfg.NW - 1))
                    else:
                        hT = io.tile([feat, 128], BF16, tag="h")
                        nc.scalar.activation(out=hT[:], in_=pw[:],
                                             func=ACT.Relu, bias=b_sb[:])
                        up = ops.tile([128, feat_out], F32, tag="up")
                        nc.tensor.matmul(out=up[:], lhsT=hT[:], rhs=w_sb[:],
                                         start=True, stop=True)
                        uo = io.tile([128, feat_out], BF16, tag="uo")
                        nc.scalar.activation(out=uo[:], in_=up[:], func=ACT.Copy)
                        nc.sync.dma_start(
                            out=out_d[ww * 128:(ww + 1) * 128, :], in_=uo[:])
            if pool:
                po = io.tile([cfg.G, feat], F32, tag="po")
                nc.vector.tensor_copy(out=po[:], in_=pool_ps[:])
                nc.sync.dma_start(out=out_d[:], in_=po[:])
    nc.compile()
    return nc


def _pad_rows(a, rows):
    out = np.zeros((rows, a.shape[1]), a.dtype)
    out[:a.shape[0]] = a
    return out


def _quad_tables(cfg, u_full):
    """Quadrant tables (interleaved: table q row r = node 4r+q), bf16."""
    return [_pad_rows(np.ascontiguousarray(u_full[i::4]), cfg.QPAD)
            for i in range(4)]


def _uown(cfg, u_full, feat, c):
    """[128, NW*feat]: partition p, cols w*feat.. = row (w*128+p) of the
    core's shard."""
    sh = _pad_rows(u_full[c * cfg.NPC:(c + 1) * cfg.NPC], cfg.NPAD)
    return np.ascontiguousarray(
        sh.reshape(cfg.NW, 128, feat).transpose(1, 0, 2).reshape(128, -1))


TRACE = False
PROFILE = []        # (name, exec_time_ns, trace_path) per launch when TRACE


def _run(nc, in_maps, name=""):
    r = run_bass_kernel_spmd(nc, in_maps, core_ids=list(range(NCORES)),
                             trace=TRACE)
    if TRACE:
        tp = r.instructions_and_trace[1] if r.instructions_and_trace else None
        PROFILE.append((name, r.exec_time_ns, tp))
    return r.results


def _agg_inputs(cfg, plan, u_full, feat):
    uqt = _quad_tables(cfg, u_full)
    maps = []
    for c in range(NCORES):
        m = {f"uq{i}": uqt[i] for i in range(4)}
        m["uown"] = _uown(cfg, u_full, feat, c)
        m["ident"] = np.eye(128, dtype=BF)
        m["iota"] = _iota_tile(128, 128).astype(BF)
        m["eidx"] = plan.idx_wrapped(c)
        m["eslot"] = plan.col_arr(plan.slot, c)
        m["ewgt"] = plan.col_arr(plan.wgt, c)
        m["enslot"] = -m["eslot"]
        m["enwgt"] = -m["ewgt"]
        maps.append(m)
    return maps


def gin_forward(cfg, x, edge_index, edge_weight, batch,
                W1, b1, W2, b2, W3, b3, ncs=None):
    src = np.asarray(edge_index[0], np.int64)
    dst = np.asarray(edge_index[1], np.int64)
    plan = Plan(cfg, src, dst, np.asarray(edge_weight, np.float32))
    if ncs is None:
        ncs = {}
    if "l1" not in ncs:
        ncs["l1"] = build_l1(cfg)
        ncs["l2"] = build_agg(cfg, plan, cfg.HID, cfg.HID, False)
        ncs["l3"] = build_agg(cfg, plan, cfg.HID, cfg.CP, False)
        ncs["l4"] = build_agg(cfg, plan, cfg.CP, None, True)

    x = np.asarray(x, np.float32)

    # L1: u1 = x @ W1
    W1b = np.asarray(W1, np.float32).astype(BF)
    maps = [{"xT": np.ascontiguousarray(
                _pad_rows(x[c * cfg.NPC:(c + 1) * cfg.NPC], cfg.NPAD)
                .T.astype(BF)),
             "W": W1b} for c in range(NCORES)]
    res = _run(ncs["l1"], maps, "l1")
    u1 = np.concatenate([res[c]["u"][:cfg.NPC] for c in range(NCORES)])

    # L2: h1 = relu(u1 + A@u1 + b1); u2 = h1 @ W2
    maps = _agg_inputs(cfg, plan, u1, cfg.HID)
    for m in maps:
        m["W"] = np.asarray(W2, np.float32).astype(BF)
        m["bvec"] = np.asarray(b1, np.float32).reshape(128, 1)
    res = _run(ncs["l2"], maps, "l2")
    u2 = np.concatenate([res[c]["u_out"][:cfg.NPC] for c in range(NCORES)])

    # L3: h2 = relu(u2 + A@u2 + b2); u3 = h2 @ W3pad
    W3p = np.zeros((cfg.HID, cfg.CP), np.float32)
    W3p[:, :cfg.C] = np.asarray(W3, np.float32)
    maps = _agg_inputs(cfg, plan, u2, cfg.HID)
    for m in maps:
        m["W"] = W3p.astype(BF)
        m["bvec"] = np.asarray(b2, np.float32).reshape(128, 1)
    res = _run(ncs["l3"], maps, "l3")
    u3 = np.concatenate([res[c]["u_out"][:cfg.NPC] for c in range(NCORES)])

    # L4: h3 = u3 + A@u3; pool (b3 folded in on host below)
    batch64 = np.asarray(batch, np.int64)
    maps = _agg_inputs(cfg, plan, u3, cfg.CP)
    for c, m in enumerate(maps):
        m["iotaG"] = _iota_tile(128, cfg.G).astype(BF)
        bv = np.full(cfg.NPAD, -1.0, np.float32)
        bv[:cfg.NPC] = batch64[c * cfg.NPC:(c + 1) * cfg.NPC].astype(np.float32)
        m["bvals"] = np.ascontiguousarray(bv.reshape(cfg.NW, 128).T)
    res = _run(ncs["l4"], maps, "l4")
    out = np.zeros((cfg.G, cfg.CP), np.float32)
    for c in range(NCORES):
        out += res[c]["pool"]
    counts = np.bincount(batch64, minlength=cfg.G).astype(np.float32)
    out = out[:, :cfg.C] + counts[:, None] * np.asarray(b3, np.float32)[None, :]
    return out.astype(np.float32)


def kernel(x, edge_index, edge_weight, batch, W1, b1, W2, b2, W3, b3):
    cfg = Cfg(N=100000, E=1600000)
    return gin_forward(cfg, x, edge_index, edge_weight, batch,
                       W1, b1, W2, b2, W3, b3)
